# revision 5
# baseline (speedup 1.0000x reference)
"""Trainium2 Bass kernel for nn_Interpolator (quadratic-form kernel interpolation).

Math (T=8192 targets, C=8192 contexts, D=64, DY=32):
    S = W + W^T
    scores[t,c] = (z_t - z_c)^T W (z_t - z_c)
                = q_tt[t] + q_cc[c] - z_t^T S z_c
      with q_tt[t] = 0.5 * z_t^T S z_t,  q_cc[c] = 0.5 * z_c^T S z_c
    theta = exp(-scores);  out = (theta @ y_context) / theta.sum(-1, keepdim)

Sharding: data-parallel over targets. Each of the 8 cores takes T/8 = 1024
targets and the full context set.

Per-core device program (in the "transposed" domain, D on partitions):
  - big matmul (K=66, fp16 operands, fp32 PSUM accumulate):
      psum[c,t] = sum_d zcT[d,c]*zsT[d,t] + (-q_cc[c])*1 + 1*(-q_tt[t])
                = -scores^T
    where zsT = S^T ztT and the augmented rows are built on device.
    fp16 (not fp32: fp32 matmuls run as 2 LOW/HIGH passes at half stream
    rate = ~4x cost; not bf16: 8-bit mantissa costs ~3e-2 absmax on the
    output, fp16's 11 bits keep it at ~5e-3 of scale).
  - theta^T = Exp(psum) on the scalar engine -> bf16 (theta spans e+-44,
    needs bf16 range; per-row constants cancel in the final normalize).
    This is the roofline engine: 8.4M exps/core at 1 elem/lane/cycle.
  - second matmul (bf16) accumulates out2T[j,t] += y_aug[c,j]*theta^T[c,t]
    over 64 context chunks; y_aug col 32 = ones gives the denominator.
Host: shard/transpose/cast inputs (layout only), concat per-core [33,1024]
outputs, divide numerator rows by the denominator row.
"""

import ml_dtypes
import numpy as np

import concourse.bacc as bacc
import concourse.bass as bass
import concourse.mybir as mybir
import concourse.tile as tile
from concourse.bass_utils import run_bass_kernel_spmd

F32 = mybir.dt.float32
F16 = mybir.dt.float16
BF16 = mybir.dt.bfloat16

T, C, D, DY = 8192, 8192, 64, 32
NCORES = 8
TL = T // NCORES          # 1024 targets per core
KAUG = D + 2              # 66 contraction rows: 64 z-dims + (-q_cc) + ones
NCHUNK = C // 128         # 64 context chunks of 128
NBLK = C // 1024          # 8 column blocks of the lhsT slab
HALF = 512                # PSUM-bank-sized matmul free dim


def _build_kernel_body(tc: tile.TileContext):
    nc = tc.nc
    Exp = mybir.ActivationFunctionType.Exp

    zt_d = nc.dram_tensor("ztt", [D, TL], F16, kind="ExternalInput")
    y_d = nc.dram_tensor("yck", [128, NCHUNK * DY], BF16, kind="ExternalInput")
    w_d = nc.dram_tensor("w", [D, D], F32, kind="ExternalInput")
    wt_d = nc.dram_tensor("wt", [D, D], F32, kind="ExternalInput")
    zc_d = [
        nc.dram_tensor(f"zct{b}", [D + 1, 1024], F16, kind="ExternalInput")
        for b in range(NBLK)
    ]
    out_d = nc.dram_tensor("out", [DY + 1, TL], F32, kind="ExternalOutput")

    with (
        tc.tile_pool(name="singles", bufs=1) as singles,
        tc.tile_pool(name="spre", bufs=3) as spre,
        tc.tile_pool(name="theta", bufs=3) as thp,
        tc.tile_pool(name="psum", bufs=2, space="PSUM") as pps,
        tc.tile_pool(name="out2", bufs=1, space="PSUM") as o2p,
    ):
        # ---- resident SBUF slabs ----
        # LC[b]: [66, 1024] fp16 lhsT slab block: rows 0..63 = zcT,
        # row 64 = -q_cc (device), row 65 = ones (host). Context order is
        # host-permuted so chunk j partition p <-> original context p*64+j.
        LC = [singles.tile([KAUG, 1024], F16, name=f"lc{b}") for b in range(NBLK)]
        RT = singles.tile([KAUG, TL], F16, name="rt")
        ZT = singles.tile([D, TL], F16, name="zt")
        YT = singles.tile([128, NCHUNK * DY], BF16, name="yt")
        YA = singles.tile([128, NCHUNK, DY + 1], BF16, name="ya")
        WS = singles.tile([D, D], F32, name="ws")
        WTT = singles.tile([D, D], F32, name="wtt")
        SSF = singles.tile([D, D], F32, name="ssf")
        SS = singles.tile([D, D], F16, name="ss")
        NH = singles.tile([D, 1], F16, name="nh")
        OSB = singles.tile([DY + 1, TL], F32, name="osb")
        TMP = singles.tile([1, TL], F16, name="tmprow")

        # ---- loads ----
        # Aug-row layout (start-partition must be 0/32/64/96 for engine ops):
        #   LC row 64 = -q_cc (scalar.copy @64), LC row 65 = ones (DMA'd)
        #   RT row 64 = ones (memset @64),       RT row 65 = -q_tt (DMA bounce)
        for b in range(NBLK):
            nc.sync.dma_start(out=LC[b][:D, :], in_=zc_d[b].ap()[:D, :])
            nc.sync.dma_start(
                out=LC[b][D + 1 : D + 2, :], in_=zc_d[b].ap()[D : D + 1, :]
            )
        nc.sync.dma_start(out=ZT, in_=zt_d.ap())
        nc.sync.dma_start(out=YT, in_=y_d.ap())
        nc.sync.dma_start(out=WS, in_=w_d.ap())
        nc.sync.dma_start(out=WTT, in_=wt_d.ap())

        nc.vector.memset(NH, -0.5)
        nc.vector.tensor_add(SSF, WS, WTT)  # S = W + W^T
        nc.vector.tensor_copy(SS, SSF)      # -> fp16

        # y_aug: [128, chunk, 33]; col 32 = 1.0 (denominator trick)
        nc.vector.tensor_copy(YA[:, :, 0:DY], YT[:, :].rearrange("p (j d) -> p j d", d=DY))
        nc.vector.memset(YA[:, :, DY : DY + 1], 1.0)

        # ---- prelude: target-side aug rows ----
        # zsT = S^T ztT  (out[d,t] = sum_d' S[d',d] * ztT[d',t])
        zs_ps = pps.tile([128, TL], F32, tag="ps")
        for h in range(TL // HALF):
            sl = slice(h * HALF, (h + 1) * HALF)
            nc.tensor.matmul(zs_ps[:D, sl], SS, ZT[:, sl], start=True, stop=True)
        nc.vector.tensor_copy(RT[:D, :], zs_ps[:D, :])  # -> fp16
        mt = spre.tile([D, TL], F16, tag="m")
        nc.vector.tensor_mul(mt, zs_ps[:D, :], ZT)
        nqt_ps = pps.tile([128, TL], F32, tag="ps")
        for h in range(TL // HALF):
            sl = slice(h * HALF, (h + 1) * HALF)
            nc.tensor.matmul(nqt_ps[0:1, sl], NH, mt[:, sl], start=True, stop=True)
        nc.scalar.copy(TMP, nqt_ps[0:1, :])                   # -q_tt (bounce @0)
        nc.sync.dma_start(out=RT[D + 1 : D + 2, :], in_=TMP)  # -> row 65
        nc.vector.memset(RT[D : D + 1, :], 1.0)               # ones row @64

        # ---- prelude: context-side -q_cc row, per 1024-block ----
        for b in range(NBLK):
            zc_blk = LC[b][:D, :]
            zs = pps.tile([128, 1024], F32, tag="ps")
            for h in range(2):
                sl = slice(h * HALF, (h + 1) * HALF)
                nc.tensor.matmul(zs[:D, sl], SS, zc_blk[:, sl], start=True, stop=True)
            mc = spre.tile([D, 1024], F16, tag="m")
            nc.vector.tensor_mul(mc, zs[:D, :], zc_blk)
            nq = pps.tile([128, 1024], F32, tag="ps")
            for h in range(2):
                sl = slice(h * HALF, (h + 1) * HALF)
                nc.tensor.matmul(nq[0:1, sl], NH, mc[:, sl], start=True, stop=True)
            nc.scalar.copy(LC[b][D : D + 1, :], nq[0:1, :])  # -q_cc row @64

        # ---- main loop over 64 context chunks ----
        o2 = o2p.tile([DY + 1, TL], F32)
        for j in range(NCHUNK):
            b, p0 = divmod(j * 128, 1024)
            lhsT = LC[b][:, p0 : p0 + 128]
            sc = pps.tile([128, TL], F32, tag="ps")
            for h in range(TL // HALF):
                sl = slice(h * HALF, (h + 1) * HALF)
                nc.tensor.matmul(sc[:, sl], lhsT, RT[:, sl], start=True, stop=True)
            th = thp.tile([128, TL], BF16)
            nc.scalar.activation(th, sc, Exp)
            for h in range(TL // HALF):
                sl = slice(h * HALF, (h + 1) * HALF)
                nc.tensor.matmul(
                    o2[:, sl],
                    YA[:, j, :],
                    th[:, sl],
                    start=(j == 0),
                    stop=(j == NCHUNK - 1),
                )

        # ---- epilogue ----
        nc.vector.tensor_copy(OSB, o2)
        nc.sync.dma_start(out=out_d.ap(), in_=OSB)


_CACHED = None


def _get_nc():
    global _CACHED
    if _CACHED is None:
        nc = bacc.Bacc(
            "TRN2",
            target_bir_lowering=False,
            debug=False,
            enable_asserts=False,
        )
        with tile.TileContext(nc) as tc:
            _build_kernel_body(tc)
        nc.compile()
        _CACHED = nc
    return _CACHED


def make_in_maps(z_context, y_context, z_target, W):
    """Host-side layout prep (transpose/reshape/cast only) + sharding."""
    z_context = np.asarray(z_context, dtype=np.float32)
    y_context = np.asarray(y_context, dtype=np.float32)
    z_target = np.asarray(z_target, dtype=np.float32)
    W = np.asarray(W, dtype=np.float32)

    # Permute contexts so chunk j partition p holds original context p*64+j;
    # this keeps both the zcT slab and the y slab DMA-contiguous.
    zcT = z_context.T.astype(np.float16)  # [64, 8192]
    # position q = j*128 + p  <-  context p*64 + j
    zc_perm = np.ascontiguousarray(
        zcT.reshape(D, 128, NCHUNK).transpose(0, 2, 1).reshape(D, C)
    )
    ones_row = np.ones((1, C), dtype=np.float16)
    zc_aug = np.concatenate([zc_perm, ones_row], axis=0)  # [65, 8192]
    zc_blocks = [
        np.ascontiguousarray(zc_aug[:, b * 1024 : (b + 1) * 1024]) for b in range(NBLK)
    ]
    # y in the same permuted order: row p of the SBUF tile holds contexts
    # p*64 + j for j in 0..63 -> plain reshape of the original y.
    yck = np.ascontiguousarray(
        y_context.reshape(128, NCHUNK * DY).astype(ml_dtypes.bfloat16)
    )
    wt = np.ascontiguousarray(W.T)

    in_maps = []
    for i in range(NCORES):
        ztT = np.ascontiguousarray(
            z_target[i * TL : (i + 1) * TL].T.astype(np.float16)
        )
        m = {"ztt": ztT, "yck": yck, "w": W, "wt": wt}
        for b in range(NBLK):
            m[f"zct{b}"] = zc_blocks[b]
        in_maps.append(m)
    return in_maps


def postprocess(results):
    """Gather per-core [33, TL] outputs -> full (T, DY) normalized output."""
    allT = np.concatenate([r["out"].T for r in results], axis=0)  # [T, 33]
    return (allT[:, :DY] / allT[:, DY : DY + 1]).astype(np.float32)


def run(in_maps, **kwargs):
    nc = _get_nc()
    return run_bass_kernel_spmd(nc, in_maps, core_ids=list(range(NCORES)), **kwargs)


def kernel(z_context, y_context, z_target, W):
    in_maps = make_in_maps(z_context, y_context, z_target, W)
    res = run(in_maps)
    return postprocess(res.results)


# revision 8
# speedup vs baseline: 1.0413x; 1.0413x over previous
"""Trainium2 Bass kernel for nn_Interpolator (quadratic-form kernel interpolation).

Math (T=8192 targets, C=8192 contexts, D=64, DY=32):
    S = W + W^T
    scores[t,c] = (z_t - z_c)^T W (z_t - z_c)
                = q_tt[t] + q_cc[c] - z_t^T S z_c
      with q_tt[t] = 0.5 * z_t^T S z_t,  q_cc[c] = 0.5 * z_c^T S z_c
    theta = exp(-scores);  out = (theta @ y_context) / theta.sum(-1, keepdim)

Sharding: data-parallel over targets. Each of the 8 cores takes T/8 = 1024
targets and the full context set.

Per-core device program (in the "transposed" domain, D on partitions):
  - big matmul (K=66, fp16 operands, fp32 PSUM accumulate):
      psum[c,t] = sum_d zcT[d,c]*zsT[d,t] + (-q_cc[c])*1 + 1*(-q_tt[t])
                = -scores^T
    where zsT = S^T ztT and the augmented rows are built on device.
    fp16 (not fp32: fp32 matmuls run as 2 LOW/HIGH passes at half stream
    rate = ~4x cost; not bf16: 8-bit mantissa costs ~3e-2 absmax on the
    output, fp16's 11 bits keep it at ~5e-3 of scale).
  - theta^T = Exp(psum) on the scalar engine -> bf16 (theta spans e+-44,
    needs bf16 range; per-row constants cancel in the final normalize).
    This is the roofline engine: 8.4M exps/core at 1 elem/lane/cycle.
  - second matmul (bf16) accumulates out2T[j,t] += y_aug[c,j]*theta^T[c,t]
    over 64 context chunks; y_aug col 32 = ones gives the denominator.
Host: shard/transpose/cast inputs (layout only), concat per-core [33,1024]
outputs, divide numerator rows by the denominator row.
"""

import ml_dtypes
import numpy as np

import concourse.bacc as bacc
import concourse.bass as bass
import concourse.mybir as mybir
import concourse.tile as tile
from concourse.bass_utils import run_bass_kernel_spmd

F32 = mybir.dt.float32
F16 = mybir.dt.float16
BF16 = mybir.dt.bfloat16

T, C, D, DY = 8192, 8192, 64, 32
NCORES = 8
TL = T // NCORES          # 1024 targets per core
KAUG = D + 2              # 66 contraction rows: 64 z-dims + (-q_cc) + ones
NCHUNK = C // 128         # 64 context chunks of 128
NBLK = C // 1024          # 8 column blocks of the lhsT slab
HALF = 512                # PSUM-bank-sized matmul free dim


def _build_kernel_body(tc: tile.TileContext):
    nc = tc.nc
    Exp = mybir.ActivationFunctionType.Exp

    zt_d = nc.dram_tensor("ztt", [D, TL], F16, kind="ExternalInput")
    y_d = nc.dram_tensor("yck", [128, NCHUNK * DY], BF16, kind="ExternalInput")
    w_d = nc.dram_tensor("w", [D, D], F32, kind="ExternalInput")
    wt_d = nc.dram_tensor("wt", [D, D], F32, kind="ExternalInput")
    zc_d = [
        nc.dram_tensor(f"zct{b}", [D + 1, 1024], F16, kind="ExternalInput")
        for b in range(NBLK)
    ]
    out_d = nc.dram_tensor("out", [DY + 1, TL], F32, kind="ExternalOutput")

    with (
        tc.tile_pool(name="singles", bufs=1) as singles,
        tc.tile_pool(name="spre", bufs=3) as spre,
        tc.tile_pool(name="theta", bufs=3) as thp,
        tc.tile_pool(name="psum", bufs=2, space="PSUM") as pps,
        tc.tile_pool(name="out2", bufs=1, space="PSUM") as o2p,
    ):
        # ---- resident SBUF slabs ----
        # LC[b]: [66, 1024] fp16 lhsT slab block: rows 0..63 = zcT,
        # row 64 = -q_cc (device), row 65 = ones (host). Context order is
        # host-permuted so chunk j partition p <-> original context p*64+j.
        LC = [singles.tile([KAUG, 1024], F16, name=f"lc{b}") for b in range(NBLK)]
        RT = singles.tile([KAUG, TL], F16, name="rt")
        ZT = singles.tile([D, TL], F16, name="zt")
        YT = singles.tile([128, NCHUNK * DY], BF16, name="yt")
        YA = singles.tile([128, NCHUNK, DY + 1], BF16, name="ya")
        WS = singles.tile([D, D], F32, name="ws")
        WTT = singles.tile([D, D], F32, name="wtt")
        SSF = singles.tile([D, D], F32, name="ssf")
        SS = singles.tile([D, D], F16, name="ss")
        NH = singles.tile([D, 1], F16, name="nh")
        OSB = singles.tile([DY + 1, TL], F32, name="osb")
        TMP = singles.tile([1, TL], F16, name="tmprow")

        WRM = singles.tile([128, HALF], BF16, name="wrm")
        EXD = singles.tile([D, 1], F32, name="exd")

        # ---- PE warm-up burst + ACT exp-table preload ----
        # HAM keeps the PE clock-gated at 1.2 GHz unless it sees a ~3.4us
        # dense-busy window; the main loop's LDW/MM + sem-wait pattern never
        # bootstraps it (measured: whole run cold). Burn ~30 dependency-free
        # matmuls at kernel start (overlapped with input DMAs) to flip HAM
        # to 8/8; the steady-state micro-idles are far below the ~3.4us MID
        # window so it stays warm. Likewise fire one dummy Exp so the ~2.7us
        # ACT table load happens during the DMA phase, not in the main loop.
        nc.vector.memset(WRM, 0.5)
        wps = pps.tile([128, HALF], F32, tag="warm", bufs=1)
        for _ in range(30):
            nc.tensor.matmul(wps, WRM[:, 0:128], WRM, start=True, stop=True)
        nc.vector.memset(EXD, 0.0)
        nc.scalar.activation(EXD, EXD, Exp)

        # ---- loads ----
        # Aug-row layout (start-partition must be 0/32/64/96 for engine ops):
        #   LC row 64 = -q_cc (copy @64),   LC row 65 = ones (DMA'd)
        #   RT row 64 = ones (memset @64),  RT row 65 = -q_tt (DMA bounce)
        # Order: small tensors feeding the prelude chain first.
        nc.sync.dma_start(out=WS, in_=w_d.ap())
        nc.sync.dma_start(out=WTT, in_=wt_d.ap())
        nc.sync.dma_start(out=ZT, in_=zt_d.ap())
        for b in range(NBLK):
            nc.sync.dma_start(out=LC[b][:D, :], in_=zc_d[b].ap()[:D, :])
            nc.sync.dma_start(
                out=LC[b][D + 1 : D + 2, :], in_=zc_d[b].ap()[D : D + 1, :]
            )
            if b == 0:
                nc.sync.dma_start(out=YT, in_=y_d.ap())

        nc.vector.memset(NH, -0.5)
        nc.vector.tensor_add(SSF, WS, WTT)  # S = W + W^T
        nc.vector.tensor_copy(SS, SSF)      # -> fp16

        # y_aug: [128, chunk, 33]; col 32 = 1.0 (denominator trick)
        nc.vector.tensor_copy(YA[:, :, 0:DY], YT[:, :].rearrange("p (j d) -> p j d", d=DY))
        nc.vector.memset(YA[:, :, DY : DY + 1], 1.0)

        # ---- prelude: target-side aug rows ----
        # zsT = S^T ztT  (out[d,t] = sum_d' S[d',d] * ztT[d',t])
        zs_ps = pps.tile([128, TL], F32, tag="ps")
        for h in range(TL // HALF):
            sl = slice(h * HALF, (h + 1) * HALF)
            nc.tensor.matmul(zs_ps[:D, sl], SS, ZT[:, sl], start=True, stop=True)
        nc.vector.tensor_copy(RT[:D, :], zs_ps[:D, :])  # -> fp16
        mt = spre.tile([D, TL], F16, tag="m")
        nc.vector.tensor_mul(mt, zs_ps[:D, :], ZT)
        nqt_ps = pps.tile([128, TL], F32, tag="ps")
        for h in range(TL // HALF):
            sl = slice(h * HALF, (h + 1) * HALF)
            nc.tensor.matmul(nqt_ps[0:1, sl], NH, mt[:, sl], start=True, stop=True)
        nc.vector.tensor_copy(TMP, nqt_ps[0:1, :])            # -q_tt (bounce @0)
        nc.sync.dma_start(out=RT[D + 1 : D + 2, :], in_=TMP)  # -> row 65
        nc.vector.memset(RT[D : D + 1, :], 1.0)               # ones row @64

        # ---- prelude: context-side -q_cc row, per 1024-block ----
        for b in range(NBLK):
            zc_blk = LC[b][:D, :]
            zs = pps.tile([128, 1024], F32, tag="ps")
            for h in range(2):
                sl = slice(h * HALF, (h + 1) * HALF)
                nc.tensor.matmul(zs[:D, sl], SS, zc_blk[:, sl], start=True, stop=True)
            mc = spre.tile([D, 1024], F16, tag="m")
            nc.vector.tensor_mul(mc, zs[:D, :], zc_blk)
            nq = pps.tile([128, 1024], F32, tag="ps")
            for h in range(2):
                sl = slice(h * HALF, (h + 1) * HALF)
                nc.tensor.matmul(nq[0:1, sl], NH, mc[:, sl], start=True, stop=True)
            nc.vector.tensor_copy(LC[b][D : D + 1, :], nq[0:1, :])  # -q_cc @64

        # ---- main loop over 64 context chunks ----
        o2 = o2p.tile([DY + 1, TL], F32)
        for j in range(NCHUNK):
            b, p0 = divmod(j * 128, 1024)
            lhsT = LC[b][:, p0 : p0 + 128]
            sc = pps.tile([128, TL], F32, tag="ps")
            for h in range(TL // HALF):
                sl = slice(h * HALF, (h + 1) * HALF)
                nc.tensor.matmul(sc[:, sl], lhsT, RT[:, sl], start=True, stop=True)
            th = thp.tile([128, TL], BF16)
            nc.scalar.activation(th, sc, Exp)
            for h in range(TL // HALF):
                sl = slice(h * HALF, (h + 1) * HALF)
                nc.tensor.matmul(
                    o2[:, sl],
                    YA[:, j, :],
                    th[:, sl],
                    start=(j == 0),
                    stop=(j == NCHUNK - 1),
                )

        # ---- epilogue ----
        nc.vector.tensor_copy(OSB, o2)
        nc.sync.dma_start(out=out_d.ap(), in_=OSB)


_CACHED = None


def _get_nc():
    global _CACHED
    if _CACHED is None:
        nc = bacc.Bacc(
            "TRN2",
            target_bir_lowering=False,
            debug=False,
            enable_asserts=False,
        )
        with tile.TileContext(nc) as tc:
            _build_kernel_body(tc)
        nc.compile()
        _CACHED = nc
    return _CACHED


def make_in_maps(z_context, y_context, z_target, W):
    """Host-side layout prep (transpose/reshape/cast only) + sharding."""
    z_context = np.asarray(z_context, dtype=np.float32)
    y_context = np.asarray(y_context, dtype=np.float32)
    z_target = np.asarray(z_target, dtype=np.float32)
    W = np.asarray(W, dtype=np.float32)

    # Permute contexts so chunk j partition p holds original context p*64+j;
    # this keeps both the zcT slab and the y slab DMA-contiguous.
    zcT = z_context.T.astype(np.float16)  # [64, 8192]
    # position q = j*128 + p  <-  context p*64 + j
    zc_perm = np.ascontiguousarray(
        zcT.reshape(D, 128, NCHUNK).transpose(0, 2, 1).reshape(D, C)
    )
    ones_row = np.ones((1, C), dtype=np.float16)
    zc_aug = np.concatenate([zc_perm, ones_row], axis=0)  # [65, 8192]
    zc_blocks = [
        np.ascontiguousarray(zc_aug[:, b * 1024 : (b + 1) * 1024]) for b in range(NBLK)
    ]
    # y in the same permuted order: row p of the SBUF tile holds contexts
    # p*64 + j for j in 0..63 -> plain reshape of the original y.
    yck = np.ascontiguousarray(
        y_context.reshape(128, NCHUNK * DY).astype(ml_dtypes.bfloat16)
    )
    wt = np.ascontiguousarray(W.T)

    in_maps = []
    for i in range(NCORES):
        ztT = np.ascontiguousarray(
            z_target[i * TL : (i + 1) * TL].T.astype(np.float16)
        )
        m = {"ztt": ztT, "yck": yck, "w": W, "wt": wt}
        for b in range(NBLK):
            m[f"zct{b}"] = zc_blocks[b]
        in_maps.append(m)
    return in_maps


def postprocess(results):
    """Gather per-core [33, TL] outputs -> full (T, DY) normalized output."""
    allT = np.concatenate([r["out"].T for r in results], axis=0)  # [T, 33]
    return (allT[:, :DY] / allT[:, DY : DY + 1]).astype(np.float32)


def run(in_maps, **kwargs):
    nc = _get_nc()
    return run_bass_kernel_spmd(nc, in_maps, core_ids=list(range(NCORES)), **kwargs)


def kernel(z_context, y_context, z_target, W):
    in_maps = make_in_maps(z_context, y_context, z_target, W)
    res = run(in_maps)
    return postprocess(res.results)


# revision 9
# speedup vs baseline: 1.1177x; 1.0733x over previous
"""Trainium2 Bass kernel for nn_Interpolator (quadratic-form kernel interpolation).

Math (T=8192 targets, C=8192 contexts, D=64, DY=32):
    S = W + W^T
    scores[t,c] = (z_t - z_c)^T W (z_t - z_c)
                = q_tt[t] + q_cc[c] - z_t^T S z_c
    theta = exp(-scores);  out = (theta @ y_context) / theta.sum(-1, keepdim)

The q_tt[t] term is a per-target constant factor exp(-q_tt[t]) on the whole
theta row, which cancels exactly in the normalization -> dropped. Only
q_cc[c] (a per-context weight) is computed.

Sharding: data-parallel over targets; each of the 8 cores takes T/8 = 1024
targets and the full context set.

Per-core device program (in the "transposed" domain, D on partitions):
  - big matmul (K=65, fp16 operands, fp32 PSUM accumulate):
      psum[c,t] = sum_d zcT[d,c]*zsT[d,t] + (-q_cc[c])*1  = cross - q_cc
    where zsT = S^T ztT. fp16, not fp32 (fp32 matmuls run as 2 LOW/HIGH
    passes at half stream rate = ~4x cost) and not bf16 (8-bit mantissa
    costs ~3e-2 absmax on the output; fp16 keeps it ~5e-3 of scale).
  - theta^T = Exp(psum) on the scalar engine -> bf16 (theta spans ~e^59,
    needs bf16 range). ACT is the roofline: 8.4M exps/core @ 1/lane/cycle.
  - second matmul (bf16) accumulates out2T[j,t] += y_aug[c,j]*theta^T[c,t]
    over 64 context chunks; y_aug col 32 = ones gives the denominator.
  - A ~5us dependency-free matmul burst at kernel start flips the PE HAM
    clock-gate to 8/8 (the main loop alone never bootstraps it), and a
    dummy Exp preloads the ACT spline table during the DMA phase.
  - The q_cc prelude (zs = S^T zc, mul, ones-reduce, row copy) is cut into
    16 half-block chains on 2 dedicated PSUM banks and emitted interleaved
    with the first 15 main-loop chunks so it rides in PE/DVE slack instead
    of serializing ahead of the loop.
Host: shard/transpose/cast inputs (layout only), concat per-core [33,1024]
outputs, divide numerator rows by the denominator row.
"""

import ml_dtypes
import numpy as np

import concourse.bacc as bacc
import concourse.bass as bass
import concourse.mybir as mybir
import concourse.tile as tile
from concourse.bass_utils import run_bass_kernel_spmd

F32 = mybir.dt.float32
F16 = mybir.dt.float16
BF16 = mybir.dt.bfloat16

T, C, D, DY = 8192, 8192, 64, 32
NCORES = 8
TL = T // NCORES          # 1024 targets per core
KAUG = D + 1              # 65 contraction rows: 64 z-dims + (-q_cc | ones)
NCHUNK = C // 128         # 64 context chunks of 128
NBLK = C // 1024          # 8 column blocks of the lhsT slab
HALF = 512                # PSUM-bank-sized matmul free dim
NWARM = 16


def _build_kernel_body(tc: tile.TileContext):
    nc = tc.nc
    Exp = mybir.ActivationFunctionType.Exp

    zt_d = nc.dram_tensor("ztt", [D, TL], F16, kind="ExternalInput")
    y_d = nc.dram_tensor("yck", [128, NCHUNK * DY], BF16, kind="ExternalInput")
    w_d = nc.dram_tensor("w", [D, D], F32, kind="ExternalInput")
    wt_d = nc.dram_tensor("wt", [D, D], F32, kind="ExternalInput")
    zc_d = [
        nc.dram_tensor(f"zct{b}", [D, 1024], F16, kind="ExternalInput")
        for b in range(NBLK)
    ]
    out_d = nc.dram_tensor("out", [DY + 1, TL], F32, kind="ExternalOutput")

    with (
        tc.tile_pool(name="singles", bufs=1) as singles,
        tc.tile_pool(name="spre", bufs=2) as spre,
        tc.tile_pool(name="theta", bufs=3) as thp,
        tc.tile_pool(name="psum", bufs=1, space="PSUM") as pps,
        tc.tile_pool(name="out2", bufs=1, space="PSUM") as o2p,
    ):
        # ---- resident SBUF slabs ----
        # LC[b]: [65, 1024] fp16 lhsT block: rows 0..63 = zcT (host-permuted:
        # chunk j partition p <-> original context p*64+j), row 64 = -q_cc.
        LC = [singles.tile([KAUG, 1024], F16, name=f"lc{b}") for b in range(NBLK)]
        RT = singles.tile([KAUG, TL], F16, name="rt")
        ZT = singles.tile([D, TL], F16, name="zt")
        YT = singles.tile([128, NCHUNK * DY], BF16, name="yt")
        YA = singles.tile([128, NCHUNK, DY + 1], BF16, name="ya")
        WS = singles.tile([D, D], F32, name="ws")
        WTT = singles.tile([D, D], F32, name="wtt")
        SSF = singles.tile([D, D], F32, name="ssf")
        SS = singles.tile([D, D], F16, name="ss")
        NH = singles.tile([D, 1], F16, name="nh")
        OSB = singles.tile([DY + 1, TL], F32, name="osb")
        WRM = singles.tile([128, HALF], BF16, name="wrm")
        EXD = singles.tile([D, 1], F32, name="exd")

        # ---- PE warm-up burst + ACT exp-table preload ----
        nc.vector.memset(WRM, 0.5)
        wps = pps.tile([128, HALF], F32, tag="pre")
        for _ in range(NWARM):
            nc.tensor.matmul(wps, WRM[:, 0:128], WRM, start=True, stop=True)
        nc.vector.memset(EXD, 0.0)
        nc.scalar.activation(EXD, EXD, Exp)

        # ---- loads (order: prelude-critical tensors first) ----
        nc.sync.dma_start(out=WS, in_=w_d.ap())
        nc.sync.dma_start(out=WTT, in_=wt_d.ap())
        nc.sync.dma_start(out=ZT, in_=zt_d.ap())
        nc.sync.dma_start(out=LC[0][:D, :], in_=zc_d[0].ap())
        nc.sync.dma_start(out=YT, in_=y_d.ap())
        for b in range(1, NBLK):
            nc.sync.dma_start(out=LC[b][:D, :], in_=zc_d[b].ap())

        nc.vector.memset(NH, -0.5)
        nc.vector.tensor_add(SSF, WS, WTT)  # S = W + W^T
        nc.vector.tensor_copy(SS, SSF)      # -> fp16

        # y_aug: [128, chunk, 33]; col 32 = 1.0 (denominator trick)
        nc.vector.tensor_copy(
            YA[:, :, 0:DY], YT[:, :].rearrange("p (j d) -> p j d", d=DY)
        )
        nc.vector.memset(YA[:, :, DY : DY + 1], 1.0)

        # ---- prelude: RT = [zsT; ones] ----
        zs_ps = pps.tile([128, TL], F32, tag="sc", bufs=2)
        for h in range(TL // HALF):
            sl = slice(h * HALF, (h + 1) * HALF)
            nc.tensor.matmul(zs_ps[:D, sl], SS, ZT[:, sl], start=True, stop=True)
        nc.vector.tensor_copy(RT[:D, :], zs_ps[:D, :])  # -> fp16
        nc.vector.memset(RT[D : D + 1, :], 1.0)         # ones row @64

        # ---- q_cc half-block chain (2 dedicated PSUM banks, bufs=1 each) ----
        def qcc_half(h):
            b, hh = divmod(h, 2)
            sl = slice(hh * HALF, (hh + 1) * HALF)
            zs = pps.tile([128, HALF], F32, tag="pre")
            nc.tensor.matmul(zs[:D, :], SS, LC[b][:D, sl], start=True, stop=True)
            mc = spre.tile([D, HALF], F16, tag="m")
            nc.vector.tensor_mul(mc, zs[:D, :], LC[b][:D, sl])
            nq = pps.tile([1, HALF], F32, tag="nq")
            nc.tensor.matmul(nq, NH, mc, start=True, stop=True)
            nc.vector.tensor_copy(LC[b][D : D + 1, sl], nq)  # -q_cc @64

        qcc_half(0)
        qcc_half(1)

        # ---- main loop over 64 context chunks ----
        o2 = o2p.tile([DY + 1, TL], F32)
        for j in range(NCHUNK):
            b, p0 = divmod(j * 128, 1024)
            lhsT = LC[b][:, p0 : p0 + 128]
            sc = pps.tile([128, TL], F32, tag="sc", bufs=2)
            for h in range(TL // HALF):
                sl = slice(h * HALF, (h + 1) * HALF)
                nc.tensor.matmul(sc[:, sl], lhsT, RT[:, sl], start=True, stop=True)
            th = thp.tile([128, TL], BF16)
            nc.scalar.activation(th, sc, Exp)
            for h in range(TL // HALF):
                sl = slice(h * HALF, (h + 1) * HALF)
                nc.tensor.matmul(
                    o2[:, sl],
                    YA[:, j, :],
                    th[:, sl],
                    start=(j == 0),
                    stop=(j == NCHUNK - 1),
                )
            if 1 <= j <= 14:
                qcc_half(j + 1)

        # ---- epilogue ----
        nc.vector.tensor_copy(OSB, o2)
        nc.sync.dma_start(out=out_d.ap(), in_=OSB)


_CACHED = None


def _get_nc():
    global _CACHED
    if _CACHED is None:
        nc = bacc.Bacc(
            "TRN2",
            target_bir_lowering=False,
            debug=False,
            enable_asserts=False,
        )
        with tile.TileContext(nc) as tc:
            _build_kernel_body(tc)
        nc.compile()
        _CACHED = nc
    return _CACHED


def make_in_maps(z_context, y_context, z_target, W):
    """Host-side layout prep (transpose/reshape/cast only) + sharding."""
    z_context = np.asarray(z_context, dtype=np.float32)
    y_context = np.asarray(y_context, dtype=np.float32)
    z_target = np.asarray(z_target, dtype=np.float32)
    W = np.asarray(W, dtype=np.float32)

    # Permute contexts so chunk j partition p holds original context p*64+j;
    # keeps both the zcT slab and the y slab DMA-contiguous.
    zcT = z_context.T.astype(np.float16)  # [64, 8192]
    # position q = j*128 + p  <-  context p*64 + j
    zc_perm = np.ascontiguousarray(
        zcT.reshape(D, 128, NCHUNK).transpose(0, 2, 1).reshape(D, C)
    )
    zc_blocks = [
        np.ascontiguousarray(zc_perm[:, b * 1024 : (b + 1) * 1024])
        for b in range(NBLK)
    ]
    # y in the same permuted order: row p of the SBUF tile holds contexts
    # p*64 + j for j in 0..63 -> plain reshape of the original y.
    yck = np.ascontiguousarray(
        y_context.reshape(128, NCHUNK * DY).astype(ml_dtypes.bfloat16)
    )
    wt = np.ascontiguousarray(W.T)

    in_maps = []
    for i in range(NCORES):
        ztT = np.ascontiguousarray(
            z_target[i * TL : (i + 1) * TL].T.astype(np.float16)
        )
        m = {"ztt": ztT, "yck": yck, "w": W, "wt": wt}
        for b in range(NBLK):
            m[f"zct{b}"] = zc_blocks[b]
        in_maps.append(m)
    return in_maps


def postprocess(results):
    """Gather per-core [33, TL] outputs -> full (T, DY) normalized output."""
    allT = np.concatenate([r["out"].T for r in results], axis=0)  # [T, 33]
    return (allT[:, :DY] / allT[:, DY : DY + 1]).astype(np.float32)


def run(in_maps, **kwargs):
    nc = _get_nc()
    return run_bass_kernel_spmd(nc, in_maps, core_ids=list(range(NCORES)), **kwargs)


def kernel(z_context, y_context, z_target, W):
    in_maps = make_in_maps(z_context, y_context, z_target, W)
    res = run(in_maps)
    return postprocess(res.results)


# revision 12
# speedup vs baseline: 1.1291x; 1.0102x over previous
"""Trainium2 Bass kernel for nn_Interpolator (quadratic-form kernel interpolation).

Math (T=8192 targets, C=8192 contexts, D=64, DY=32):
    S = W + W^T
    scores[t,c] = (z_t - z_c)^T W (z_t - z_c)
                = q_tt[t] + q_cc[c] - z_t^T S z_c
    theta = exp(-scores);  out = (theta @ y_context) / theta.sum(-1, keepdim)

The q_tt[t] term is a per-target constant factor exp(-q_tt[t]) on the whole
theta row, which cancels exactly in the normalization -> dropped. Only
q_cc[c] (a per-context weight) is computed.

Sharding: data-parallel over targets; each of the 8 cores takes T/8 = 1024
targets and the full context set.

Per-core device program (in the "transposed" domain, D on partitions):
  - big matmul (K=65, fp16 operands, fp32 PSUM accumulate):
      psum[c,t] = sum_d zcT[d,c]*zsT[d,t] + (-q_cc[c])*1  = cross - q_cc
    where zsT = S^T ztT. fp16, not fp32 (fp32 matmuls run as 2 LOW/HIGH
    passes at half stream rate = ~4x cost) and not bf16 (8-bit mantissa
    costs ~3e-2 absmax on the output; fp16 keeps it ~5e-3 of scale).
  - theta^T = Exp(psum) on the scalar engine -> bf16 (theta spans ~e^59,
    needs bf16 range). ACT is the roofline: 8.4M exps/core @ 1/lane/cycle.
  - second matmul (bf16) accumulates out2T[j,t] += y_aug[c,j]*theta^T[c,t]
    over 64 context chunks; y_aug col 32 = ones gives the denominator.
  - A ~5us dependency-free matmul burst at kernel start flips the PE HAM
    clock-gate to 8/8 (the main loop alone never bootstraps it), and a
    dummy Exp preloads the ACT spline table during the DMA phase.
  - The q_cc prelude (zs = S^T zc, mul, ones-reduce, row copy) is cut into
    16 half-block chains on 2 dedicated PSUM banks and emitted interleaved
    with the first 15 main-loop chunks so it rides in PE/DVE slack instead
    of serializing ahead of the loop.
Host: shard/transpose/cast inputs (layout only), concat per-core [33,1024]
outputs, divide numerator rows by the denominator row.
"""

import ml_dtypes
import numpy as np

import concourse.bacc as bacc
import concourse.bass as bass
import concourse.mybir as mybir
import concourse.tile as tile
from concourse.bass_utils import run_bass_kernel_spmd

F32 = mybir.dt.float32
F16 = mybir.dt.float16
BF16 = mybir.dt.bfloat16

T, C, D, DY = 8192, 8192, 64, 32
NCORES = 8
TL = T // NCORES          # 1024 targets per core
KAUG = D + 1              # 65 contraction rows: 64 z-dims + (-q_cc | ones)
NCHUNK = C // 128         # 64 context chunks of 128
NBLK = C // 1024          # 8 column blocks of the lhsT slab
HALF = 512                # PSUM-bank-sized matmul free dim
NWARM = 16


def _build_kernel_body(tc: tile.TileContext):
    nc = tc.nc
    Exp = mybir.ActivationFunctionType.Exp

    zt_d = nc.dram_tensor("ztt", [D, TL], F16, kind="ExternalInput")
    y_d = [
        nc.dram_tensor(f"yck{i}", [128, NCHUNK * DY // 2], BF16, kind="ExternalInput")
        for i in range(2)
    ]
    w_d = nc.dram_tensor("w", [D, D], F32, kind="ExternalInput")
    wt_d = nc.dram_tensor("wt", [D, D], F32, kind="ExternalInput")
    zc_d = [
        nc.dram_tensor(f"zct{b}", [D, 1024], F16, kind="ExternalInput")
        for b in range(NBLK)
    ]
    out_d = nc.dram_tensor("out", [DY + 1, TL], F32, kind="ExternalOutput")

    with (
        tc.tile_pool(name="singles", bufs=1) as singles,
        tc.tile_pool(name="spre", bufs=2) as spre,
        tc.tile_pool(name="theta", bufs=3) as thp,
        tc.tile_pool(name="psum", bufs=1, space="PSUM") as pps,
        tc.tile_pool(name="out2", bufs=1, space="PSUM") as o2p,
    ):
        # ---- resident SBUF slabs ----
        # LC[b]: [65, 1024] fp16 lhsT block: rows 0..63 = zcT (host-permuted:
        # chunk j partition p <-> original context p*64+j), row 64 = -q_cc.
        LC = [singles.tile([KAUG, 1024], F16, name=f"lc{b}") for b in range(NBLK)]
        RT = singles.tile([KAUG, TL], F16, name="rt")
        ZT = singles.tile([D, TL], F16, name="zt")
        YT = singles.tile([128, NCHUNK * DY], BF16, name="yt")
        YA = singles.tile([128, NCHUNK, DY + 1], BF16, name="ya")
        WS = singles.tile([D, D], F32, name="ws")
        WTT = singles.tile([D, D], F32, name="wtt")
        SSF = singles.tile([D, D], F32, name="ssf")
        SS = singles.tile([D, D], F16, name="ss")
        NH = singles.tile([D, 1], F16, name="nh")
        OSB = singles.tile([DY + 1, TL], F32, name="osb")
        WRM = singles.tile([128, HALF], BF16, name="wrm")
        EXD = singles.tile([D, 1], F32, name="exd")

        # ---- PE warm-up burst + ACT exp-table preload ----
        # Filler matmuls (dependency-free, own PSUM bank) bridge every
        # PE-idle window in the head so HAM warms early and never sees a
        # ~3.4us MID window before the main loop's dense stream takes over.
        wps = pps.tile([128, HALF], F32, tag="warm")

        def fill(n):
            for _ in range(n):
                nc.tensor.matmul(wps, WRM[:, 0:128], WRM, start=True, stop=True)

        nc.vector.memset(WRM, 0.5)
        fill(NWARM)
        nc.vector.memset(EXD, 0.0)
        nc.scalar.activation(EXD, EXD, Exp)

        # ---- loads (order: prelude-critical tensors first) ----
        nc.sync.dma_start(out=WS, in_=w_d.ap())
        nc.sync.dma_start(out=WTT, in_=wt_d.ap())
        nc.sync.dma_start(out=ZT, in_=zt_d.ap())
        nc.sync.dma_start(out=LC[0][:D, :], in_=zc_d[0].ap())
        half_y = NCHUNK * DY // 2
        nc.sync.dma_start(out=YT[:, :half_y], in_=y_d[0].ap())
        nc.sync.dma_start(out=YT[:, half_y:], in_=y_d[1].ap())
        for b in range(1, NBLK):
            nc.sync.dma_start(out=LC[b][:D, :], in_=zc_d[b].ap())

        # DVE emission order matters: the queue is strict FIFO, so emit in
        # expected-readiness order (an op waiting on a slow DMA would
        # head-of-line-block everything behind it).
        nc.vector.tensor_add(SSF, WS, WTT)  # S = W + W^T
        nc.vector.tensor_copy(SS, SSF)      # -> fp16
        nc.vector.memset(NH, -0.5)
        nc.vector.memset(RT[D : D + 1, :], 1.0)  # ones row @64

        # ---- prelude: RT = [zsT; ones] ----
        zs_ps = pps.tile([128, TL], F32, tag="sc", bufs=2)
        for h in range(TL // HALF):
            sl = slice(h * HALF, (h + 1) * HALF)
            nc.tensor.matmul(zs_ps[:D, sl], SS, ZT[:, sl], start=True, stop=True)
        nc.vector.tensor_copy(RT[:D, :], zs_ps[:D, :])  # -> fp16

        # ---- q_cc half-block chain ----
        # One [128, HALF] PSUM tile per half: zs lands in partitions 0..63,
        # the ones-reduce (-q_cc) in partition 64 of the same bank; pool
        # slot reuse (bufs=1) serializes bank hand-off safely.
        def qcc_half(h, nfill=0):
            b, hh = divmod(h, 2)
            sl = slice(hh * HALF, (hh + 1) * HALF)
            ps = pps.tile([128, HALF], F32, tag="pre")
            nc.tensor.matmul(ps[:D, :], SS, LC[b][:D, sl], start=True, stop=True)
            if nfill:
                fill(nfill)
            mc = spre.tile([D, HALF], F16, tag="m")
            nc.vector.tensor_mul(mc, ps[:D, :], LC[b][:D, sl])
            nc.tensor.matmul(ps[D : D + 1, :], NH, mc, start=True, stop=True)
            nc.vector.tensor_copy(LC[b][D : D + 1, sl], ps[D : D + 1, :])

        qcc_half(0, nfill=4)
        qcc_half(1, nfill=4)
        fill(4)

        # y_aug: [128, chunk, 33]; col 32 = 1.0 (denominator trick).
        # Emitted late: waits on the big y DMA, must not block the q_cc ops.
        nc.vector.tensor_copy(
            YA[:, :, 0:DY], YT[:, :].rearrange("p (j d) -> p j d", d=DY)
        )
        nc.vector.memset(YA[:, :, DY : DY + 1], 1.0)

        # ---- main loop over 64 context chunks ----
        o2 = o2p.tile([DY + 1, TL], F32)
        for j in range(NCHUNK):
            b, p0 = divmod(j * 128, 1024)
            lhsT = LC[b][:, p0 : p0 + 128]
            sc = pps.tile([128, TL], F32, tag="sc", bufs=2)
            for h in range(TL // HALF):
                sl = slice(h * HALF, (h + 1) * HALF)
                nc.tensor.matmul(sc[:, sl], lhsT, RT[:, sl], start=True, stop=True)
            th = thp.tile([128, TL], BF16)
            nc.scalar.activation(th, sc, Exp)
            for h in range(TL // HALF):
                sl = slice(h * HALF, (h + 1) * HALF)
                nc.tensor.matmul(
                    o2[:, sl],
                    YA[:, j, :],
                    th[:, sl],
                    start=(j == 0),
                    stop=(j == NCHUNK - 1),
                )
            if 1 <= j <= 14:
                qcc_half(j + 1)

        # ---- epilogue ----
        nc.vector.tensor_copy(OSB, o2)
        nc.sync.dma_start(out=out_d.ap(), in_=OSB)


_CACHED = None


def _get_nc():
    global _CACHED
    if _CACHED is None:
        nc = bacc.Bacc(
            "TRN2",
            target_bir_lowering=False,
            debug=False,
            enable_asserts=False,
        )
        with tile.TileContext(nc) as tc:
            _build_kernel_body(tc)
        nc.compile()
        _CACHED = nc
    return _CACHED


def make_in_maps(z_context, y_context, z_target, W):
    """Host-side layout prep (transpose/reshape/cast only) + sharding."""
    z_context = np.asarray(z_context, dtype=np.float32)
    y_context = np.asarray(y_context, dtype=np.float32)
    z_target = np.asarray(z_target, dtype=np.float32)
    W = np.asarray(W, dtype=np.float32)

    # Permute contexts so chunk j partition p holds original context p*64+j;
    # keeps both the zcT slab and the y slab DMA-contiguous.
    zcT = z_context.T.astype(np.float16)  # [64, 8192]
    # position q = j*128 + p  <-  context p*64 + j
    zc_perm = np.ascontiguousarray(
        zcT.reshape(D, 128, NCHUNK).transpose(0, 2, 1).reshape(D, C)
    )
    zc_blocks = [
        np.ascontiguousarray(zc_perm[:, b * 1024 : (b + 1) * 1024])
        for b in range(NBLK)
    ]
    # y in the same permuted order: row p of the SBUF tile holds contexts
    # p*64 + j for j in 0..63 -> plain reshape of the original y.
    yck = y_context.reshape(128, NCHUNK * DY).astype(ml_dtypes.bfloat16)
    half_y = NCHUNK * DY // 2
    yck0 = np.ascontiguousarray(yck[:, :half_y])
    yck1 = np.ascontiguousarray(yck[:, half_y:])
    wt = np.ascontiguousarray(W.T)

    in_maps = []
    for i in range(NCORES):
        ztT = np.ascontiguousarray(
            z_target[i * TL : (i + 1) * TL].T.astype(np.float16)
        )
        m = {"ztt": ztT, "yck0": yck0, "yck1": yck1, "w": W, "wt": wt}
        for b in range(NBLK):
            m[f"zct{b}"] = zc_blocks[b]
        in_maps.append(m)
    return in_maps


def postprocess(results):
    """Gather per-core [33, TL] outputs -> full (T, DY) normalized output."""
    allT = np.concatenate([r["out"].T for r in results], axis=0)  # [T, 33]
    return (allT[:, :DY] / allT[:, DY : DY + 1]).astype(np.float32)


def run(in_maps, **kwargs):
    nc = _get_nc()
    return run_bass_kernel_spmd(nc, in_maps, core_ids=list(range(NCORES)), **kwargs)


def kernel(z_context, y_context, z_target, W):
    in_maps = make_in_maps(z_context, y_context, z_target, W)
    res = run(in_maps)
    return postprocess(res.results)
